# revision 1
# baseline (speedup 1.0000x reference)
"""Trainium2 Bass kernel for nn_DeepImageAnalogy (patchmatch + reconstruction).

Self-contained: builds an 8-core SPMD Bass/Tile program on first call and
caches it. kernel(**inputs) takes full inputs, shards rows across 8 cores,
runs on hardware, and reassembles full outputs (rec_s, rec_t, similarity).
"""
import sys
for p in ('/opt/trn_rl_repo', '/root/.axon_site/_ro/trn_rl_repo'):
    if p not in sys.path:
        sys.path.insert(0, p)
import numpy as np

import concourse.bass as bass
import concourse.bacc as bacc
import concourse.mybir as mybir
import concourse.tile as tile
from concourse.bass_utils import run_bass_kernel_spmd

dt = mybir.dt
OP = mybir.AluOpType
N_CORES = 8
H = W = 128
ROWS = 16
ITERS, RADIUS = 5, 4
PADW = 130
NPAD = PADW * PADW + 1
C04 = float(np.float32(1.0) - np.float32(0.6))
RBIAS = 8388608.0

# ---------------------------------------------------------------- host prep

def _pad_hwc(feat_chw):
    hwc = np.ascontiguousarray(np.transpose(feat_chw, (1, 2, 0)))
    p = np.pad(hwc, ((1, 1), (1, 1), (0, 0)), mode="edge").reshape(PADW * PADW, -1)
    z = np.zeros((1, p.shape[1]), p.dtype)
    return np.ascontiguousarray(np.concatenate([p, z], 0)).astype(np.float32)


def _rand_slab(rand, k):
    r = rand[:, :, 0, :, k * ROWS:(k + 1) * ROWS, :]
    out = np.empty((ITERS, RADIUS, 128, 2 * ROWS), np.float32)
    out[:, :, :, 0:ROWS] = np.transpose(r[:, :, 0], (0, 1, 3, 2))
    out[:, :, :, ROWS:] = np.transpose(r[:, :, 1], (0, 1, 3, 2))
    return np.ascontiguousarray(out)


def _nnf_slab(init01, k):
    n = init01[0].astype(np.float32) * np.float32(H - 1)
    out = np.empty((128, 2 * ROWS), np.float32)
    out[:, 0:ROWS] = n[0, k * ROWS:(k + 1) * ROWS, :].T
    out[:, ROWS:] = n[1, k * ROWS:(k + 1) * ROWS, :].T
    return np.ascontiguousarray(out)


def _make_hsel(k):
    hs = np.zeros((32, 4), np.float32)
    if k > 0:
        hs[4 * (k - 1) + 1, 0] = 1.0
        hs[4 * (k - 1) + 3, 2] = 1.0
    if k < N_CORES - 1:
        hs[4 * (k + 1) + 0, 1] = 1.0
        hs[4 * (k + 1) + 2, 3] = 1.0
    return hs


def _make_invcnt(k):
    cy = np.full(H, 3.0, np.float32); cy[0] = 2.0; cy[-1] = 2.0
    cnt = cy[:, None] * cy[None, :]
    inv = np.float32(0.6) / (cnt + np.float32(1e-8))
    return np.ascontiguousarray(inv[k * ROWS:(k + 1) * ROWS, :].T)


def _make_mf(k):
    M = np.zeros((128, 9, ROWS), np.float32)
    Fp = np.zeros((128, 9, ROWS), np.float32)
    for v in range(128):
        for A in range(9):
            dy, dx = A // 3, A % 3
            for u in range(ROWS):
                i_g = 16 * k + u - (dy - 1)
                j = v - (dx - 1)
                if (0 <= i_g < H) and (0 <= j < W):
                    M[v, A, u] = 1.0
                    Fp[v, A, u] = dy * 130 + dx
                else:
                    Fp[v, A, u] = PADW * PADW
    return (np.ascontiguousarray(M.reshape(128, -1)),
            np.ascontiguousarray(Fp.reshape(128, -1)))


def _make_band():
    A = np.zeros((128, 128), np.float32)
    for j in range(128):
        for d in (-1, 0, 1):
            A[min(max(j + d, 0), 127), j] += 1.0
    return A


def _make_shift(direction):
    P = np.zeros((128, 128), np.float32)
    for i in range(128):
        P[(i - direction) % 128, i] = 1.0
    return P


def _make_m16():
    p = np.arange(128)[:, None]
    kk = np.arange(512)[None, :]
    return ((kk % 16) == (p % 16)).astype(np.float32)


# ---------------------------------------------------------------- program

def _build():
    nc = bacc.Bacc("TRN2", target_bir_lowering=False, debug=False, num_devices=N_CORES)
    f32, i16, u8 = dt.float32, dt.int16, dt.uint8

    s_slab = nc.dram_tensor("s_slab", [256, ROWS, 128], f32, kind="ExternalInput")
    t_slab = nc.dram_tensor("t_slab", [256, ROWS, 128], f32, kind="ExternalInput")
    tpad = nc.dram_tensor("tpad", [NPAD, 256], f32, kind="ExternalInput")
    spad = nc.dram_tensor("spad", [NPAD, 256], f32, kind="ExternalInput")
    nnf_f_in = nc.dram_tensor("nnf_f", [128, 32], f32, kind="ExternalInput")
    nnf_b_in = nc.dram_tensor("nnf_b", [128, 32], f32, kind="ExternalInput")
    rand_f_in = nc.dram_tensor("rand_f", [ITERS, RADIUS, 128, 32], f32, kind="ExternalInput")
    rand_b_in = nc.dram_tensor("rand_b", [ITERS, RADIUS, 128, 32], f32, kind="ExternalInput")
    hsel_in = nc.dram_tensor("hsel", [32, 4], f32, kind="ExternalInput")
    minp_in = nc.dram_tensor("minp", [128, 144], f32, kind="ExternalInput")
    finp_in = nc.dram_tensor("finp", [128, 144], f32, kind="ExternalInput")
    invcnt_in = nc.dram_tensor("invcnt", [128, ROWS], f32, kind="ExternalInput")
    band_in = nc.dram_tensor("band", [128, 128], f32, kind="ExternalInput")
    shp_in = nc.dram_tensor("shp", [128, 128], f32, kind="ExternalInput")
    shm_in = nc.dram_tensor("shm", [128, 128], f32, kind="ExternalInput")
    m16_in = nc.dram_tensor("m16", [128, 512], f32, kind="ExternalInput")
    ident_in = nc.dram_tensor("ident", [128, 128], f32, kind="ExternalInput")

    rec_s_out = nc.dram_tensor("rec_s", [256, ROWS, 128], f32, kind="ExternalOutput")
    rec_t_out = nc.dram_tensor("rec_t", [256, ROWS, 128], f32, kind="ExternalOutput")
    sim_out = nc.dram_tensor("sim", [ROWS, 128], f32, kind="ExternalOutput")

    d_bounce = nc.dram_tensor("d_bounce", [ROWS, 128], f32)
    d_full = nc.dram_tensor("d_full", [128, 128], f32)
    ct_dram = nc.dram_tensor("ct_dram", [128, 128], f32)
    bnd_bounce = nc.dram_tensor("bnd_bounce", [4, 128], f32)
    bnd_full = nc.dram_tensor("bnd_full", [32, 128], f32)
    halo_dram = nc.dram_tensor("halo_dram", [4, 128], f32)

    RG = [list(range(N_CORES))]

    with tile.TileContext(nc) as tc:
        with tc.tile_pool(name="state", bufs=1) as statep:
            ones = statep.tile([128, 1], f32, tag="ones", name="ones")
            nc.vector.memset(ones[:, :], 1.0)
            band_sb = statep.tile([128, 128], f32, tag="band", name="band")
            ident_sb = statep.tile([128, 128], f32, tag="ident", name="ident")
            shp_sb = statep.tile([128, 128], f32, tag="shp", name="shp")
            shm_sb = statep.tile([128, 128], f32, tag="shm", name="shm")
            nc.sync.dma_start(band_sb[:, :], band_in[:, :])
            nc.sync.dma_start(ident_sb[:, :], ident_in[:, :])
            nc.sync.dma_start(shp_sb[:, :], shp_in[:, :])
            nc.sync.dma_start(shm_sb[:, :], shm_in[:, :])

            # ---------------- phase 1: per-pixel feature distance D ----------------
            with (
                tc.tile_pool(name="ph1", bufs=2) as ph1,
                tc.tile_pool(name="ph1ps", bufs=1, space="PSUM") as ph1ps,
                tc.tile_pool(name="ph1seq", bufs=1) as ph1seq,
            ):
                dps = [ph1ps.tile([1, 512], f32, tag=f"dp{j}", name=f"dp{j}") for j in range(4)]
                for half in range(2):
                    for quarter in range(2):
                        sh = ph1.tile([128, 1024], f32, tag="sh", name="sh")
                        th = ph1.tile([128, 1024], f32, tag="th", name="th")
                        sl = slice(128 * half, 128 * (half + 1))
                        fsl = slice(1024 * quarter, 1024 * (quarter + 1))
                        nc.sync.dma_start(sh[:, :], s_slab[sl, :, :].rearrange("c r w -> c (r w)")[:, fsl])
                        nc.sync.dma_start(th[:, :], t_slab[sl, :, :].rearrange("c r w -> c (r w)")[:, fsl])
                        nc.vector.tensor_tensor(sh[:, :], sh[:, :], th[:, :], op=OP.subtract)
                        nc.scalar.square(th[:, :], sh[:, :])
                        for j in range(2):
                            nc.tensor.matmul(dps[2 * quarter + j][:, :], ones[:, :],
                                             th[:, 512 * j:512 * (j + 1)],
                                             start=(half == 0), stop=(half == 1))
                dsl = ph1seq.tile([1, 2048], f32, tag="dsl", name="dsl")
                for j in range(4):
                    nc.vector.tensor_copy(dsl[:, 512 * j:512 * (j + 1)], dps[j][:, :])
                nc.sync.dma_start(bass.AP(d_bounce.ap().tensor, 0, [[2048, 1], [1, 2048]]), dsl[:, :])
            nc.gpsimd.collective_compute("AllGather", OP.bypass, replica_groups=RG,
                                         ins=[d_bounce.ap().opt()], outs=[d_full.ap().opt()])

            # ---------------- phase 2: window-cost table C (transposed) ----------------
            with (
                tc.tile_pool(name="ph2", bufs=1) as ph2,
                tc.tile_pool(name="ph2ps", bufs=2, space="PSUM") as psp2,
            ):
                dful = ph2.tile([128, 128], f32, tag="dful", name="dful")
                nc.sync.dma_start(dful[:, :], d_full[:, :])
                m1p = psp2.tile([128, 128], f32, tag="cmm", name="cmm1")
                nc.tensor.matmul(m1p[:, :], band_sb[:, :], dful[:, :], start=True, stop=True)
                m1s = ph2.tile([128, 128], f32, tag="m1s", name="m1s")
                nc.vector.tensor_copy(m1s[:, :], m1p[:, :])
                m1tp = psp2.tile([128, 128], f32, tag="cmm", name="cmm2")
                nc.tensor.transpose(m1tp[:, :], m1s[:, :], ident_sb[:, :])
                m1ts = ph2.tile([128, 128], f32, tag="m1ts", name="m1ts")
                nc.vector.tensor_copy(m1ts[:, :], m1tp[:, :])
                ctp = psp2.tile([128, 128], f32, tag="cmm", name="cmm3")
                nc.tensor.matmul(ctp[:, :], band_sb[:, :], m1ts[:, :], start=True, stop=True)
                cts = ph2.tile([128, 128], f32, tag="cts", name="cts")
                nc.vector.tensor_copy(cts[:, :], ctp[:, :])
                nc.sync.dma_start(ct_dram[:, :], cts[:, :])

            # ---------------- phase 3: patchmatch, phase 4: reconstruction ----------------
            with (
                tc.tile_pool(name="big", bufs=1) as bigp,
                tc.tile_pool(name="work", bufs=2) as workp,
                tc.tile_pool(name="seq", bufs=1) as seqp,
                tc.tile_pool(name="ps34", bufs=2, space="PSUM") as psp,
            ):
                crep = bigp.tile([128, 16384], f32, tag="crep", name="crep")
                nc.sync.dma_start(crep[:, :], bass.AP(ct_dram.ap().tensor, 0, [[0, 128], [1, 16384]]))
                m16s = statep.tile([128, 512], f32, tag="m16s", name="m16s")
                nc.sync.dma_start(m16s[:, :], m16_in[:, :])
                nnf = {"f": statep.tile([128, 32], f32, tag="nnf_f_t", name="nnf_f_t"),
                       "b": statep.tile([128, 32], f32, tag="nnf_b_t", name="nnf_b_t")}
                nc.sync.dma_start(nnf["f"][:, :], nnf_f_in[:, :])
                nc.sync.dma_start(nnf["b"][:, :], nnf_b_in[:, :])
                dcur = statep.tile([128, 32], f32, tag="dcur", name="dcur")

                def eval_pair(cand_f, cand_b, dd_out):
                    rdp = workp.tile([128, 32], f32, tag="rdp", name="rdp")
                    rdq = workp.tile([128, 32], f32, tag="rdq", name="rdq")
                    nc.vector.tensor_scalar(rdp[:, :], cand_f[:, :], RBIAS, RBIAS, op0=OP.add, op1=OP.subtract)
                    nc.vector.tensor_scalar(rdq[:, :], cand_b[:, :], RBIAS, RBIAS, op0=OP.add, op1=OP.subtract)
                    idxf = workp.tile([128, 32], f32, tag="idxf", name="idxf")
                    nc.vector.scalar_tensor_tensor(idxf[:, 0:16], rdp[:, 16:32], 128.0, rdp[:, 0:16], OP.mult, OP.add)
                    nc.vector.scalar_tensor_tensor(idxf[:, 16:32], rdq[:, 16:32], 128.0, rdq[:, 0:16], OP.mult, OP.add)
                    idx16 = workp.tile([128, 32], i16, tag="idx16", name="idx16")
                    nc.vector.tensor_copy(idx16[:, :], idxf[:, :])
                    g = workp.tile([128, 512], f32, tag="gev", name="gev")
                    nc.gpsimd.ap_gather(g[:, :], crep[:, :], idx16[:, :],
                                        channels=128, num_elems=16384, d=1, num_idxs=512)
                    nc.vector.tensor_tensor(g[:, :], g[:, :], m16s[:, :], op=OP.mult)
                    nc.vector.tensor_reduce(dd_out[:, :], g[:, :].rearrange("p (s j) -> p s j", j=16),
                                            axis=mybir.AxisListType.X, op=OP.add)

                eval_pair(nnf["f"], nnf["b"], dcur)

                def prop(direction):
                    psh = shp_sb if direction == 1 else shm_sb
                    for di, dk in enumerate("fb"):
                        dcs = dcur[:, 16 * di:16 * (di + 1)]
                        nshp = psp.tile([128, 32], f32, tag="nshp", name=f"nshp{dk}")
                        dshp = psp.tile([128, 16], f32, tag="dshp", name=f"dshp{dk}")
                        nc.tensor.matmul(nshp[:, :], psh[:, :], nnf[dk][:, :], start=True, stop=True)
                        nc.tensor.matmul(dshp[:, :], psh[:, :], dcs, start=True, stop=True)
                        m = workp.tile([128, 16], u8, tag=f"mp{dk}", name=f"mp{dk}")
                        nc.vector.tensor_tensor(m[:, :], dshp[:, :], dcs, op=OP.is_lt)
                        nc.vector.copy_predicated(nnf[dk][:, 0:16], m[:, :], nshp[:, 0:16])
                        nc.vector.copy_predicated(nnf[dk][:, 16:32], m[:, :], nshp[:, 16:32])
                        nc.vector.tensor_tensor(dcs, dcs, dshp[:, :], op=OP.min)

                for it in range(ITERS):
                    prop(1)
                    prop(-1)
                    for i in range(RADIUS):
                        scale = float(2.0 ** (-i))
                        rn = {}
                        for dk, rin in (("f", rand_f_in), ("b", rand_b_in)):
                            rt = workp.tile([128, 32], f32, tag=f"rt{dk}", name=f"rt{dk}")
                            nc.sync.dma_start(rt[:, :], rin[it, i, :, :])
                            r = workp.tile([128, 32], f32, tag=f"rn{dk}", name=f"rn{dk}")
                            nc.vector.scalar_tensor_tensor(r[:, :], rt[:, :], scale, nnf[dk][:, :], OP.mult, OP.add)
                            nc.vector.tensor_scalar(r[:, :], r[:, :], 0.0, 127.0, op0=OP.max, op1=OP.min)
                            rn[dk] = r
                        dd = workp.tile([128, 32], f32, tag="ddrs", name="ddrs")
                        eval_pair(rn["f"], rn["b"], dd)
                        for di, dk in enumerate("fb"):
                            m = workp.tile([128, 16], u8, tag=f"mr{dk}", name=f"mr{dk}")
                            dslice = dcur[:, 16 * di:16 * (di + 1)]
                            nc.vector.tensor_tensor(m[:, :], dd[:, 16 * di:16 * (di + 1)], dslice, op=OP.is_lt)
                            nc.vector.copy_predicated(nnf[dk][:, 0:16], m[:, :], rn[dk][:, 0:16])
                            nc.vector.copy_predicated(nnf[dk][:, 16:32], m[:, :], rn[dk][:, 16:32])
                            nc.vector.tensor_tensor(dslice, dslice, dd[:, 16 * di:16 * (di + 1)], op=OP.min)

                # similarity = -0.5*(d_fwd + d_bwd)
                simt = seqp.tile([128, 16], f32, tag="simt", name="simt")
                nc.vector.tensor_tensor(simt[:, :], dcur[:, 0:16], dcur[:, 16:32], op=OP.add)
                nc.vector.tensor_scalar(simt[:, :], simt[:, :], -0.5, None, op0=OP.mult)
                simp = psp.tile([16, 128], f32, tag="mis1", name="simp", bufs=1)
                nc.tensor.transpose(simp[:, :], simt[:, :], ident_sb[:, :])
                sims = seqp.tile([16, 128], f32, tag="sims", name="sims")
                nc.vector.tensor_copy(sims[:, :], simp[:, :])
                nc.sync.dma_start(sim_out[:, :], sims[:, :])

                # rounded planes + boundary halo exchange
                rdf = seqp.tile([128, 32], f32, tag="rdf", name="rdf")
                nc.vector.tensor_scalar(rdf[:, :], nnf["f"][:, :], RBIAS, RBIAS, op0=OP.add, op1=OP.subtract)
                tnp = psp.tile([32, 128], f32, tag="mis1", name="tnp", bufs=1)
                nc.tensor.transpose(tnp[:, :], rdf[:, :], ident_sb[:, :])
                tns = seqp.tile([32, 128], f32, tag="tns", name="tns")
                nc.vector.tensor_copy(tns[:, :], tnp[:, :])
                nc.sync.dma_start(bnd_bounce[0:1, :], tns[0:1, :])
                nc.sync.dma_start(bnd_bounce[1:2, :], tns[15:16, :])
                nc.sync.dma_start(bnd_bounce[2:3, :], tns[16:17, :])
                nc.sync.dma_start(bnd_bounce[3:4, :], tns[31:32, :])
                nc.gpsimd.collective_compute("AllGather", OP.bypass, replica_groups=RG,
                                             ins=[bnd_bounce.ap().opt()], outs=[bnd_full.ap().opt()])
                bnds = seqp.tile([32, 128], f32, tag="bnds", name="bnds")
                hsels = seqp.tile([32, 4], f32, tag="hsels", name="hsels")
                nc.sync.dma_start(bnds[:, :], bnd_full[:, :])
                nc.sync.dma_start(hsels[:, :], hsel_in[:, :])
                halop = psp.tile([4, 128], f32, tag="mis1", name="halop", bufs=1)
                nc.tensor.matmul(halop[:, :], hsels[:, :], bnds[:, :], start=True, stop=True)
                halos = seqp.tile([4, 128], f32, tag="halos", name="halos")
                nc.vector.tensor_copy(halos[:, :], halop[:, :])
                nc.sync.dma_start(halo_dram[:, :], halos[:, :])

                invcs = statep.tile([128, ROWS], f32, tag="invcs", name="invcs")
                nc.sync.dma_start(invcs[:, :], invcnt_in[:, :])
                ms_t = seqp.tile([128, 144], f32, tag="ms_t", name="ms_t")
                fs_t = seqp.tile([128, 144], f32, tag="fs_t", name="fs_t")
                nc.sync.dma_start(ms_t[:, :], minp_in[:, :])
                nc.sync.dma_start(fs_t[:, :], finp_in[:, :])

                # NNF coordinate planes (rows u-1..u+16) and +-1 column shifts
                planes = {}
                for pi, lbl in ((0, "y"), (1, "x")):
                    pv = seqp.tile([128, 18], f32, tag=f"pv{lbl}", name=f"pv{lbl}")
                    nc.vector.tensor_copy(pv[:, 1:17], rdf[:, 16 * pi:16 * pi + 16])
                    nc.sync.dma_start(pv[:, 0:1],
                                      bass.AP(halo_dram.ap().tensor, 2 * pi * 128, [[1, 128], [1, 1]]))
                    nc.sync.dma_start(pv[:, 17:18],
                                      bass.AP(halo_dram.ap().tensor, (2 * pi + 1) * 128, [[1, 128], [1, 1]]))
                    planes[lbl + "c"] = pv
                    for sm, sl in ((shp_sb, "p"), (shm_sb, "m")):
                        pp = psp.tile([128, 18], f32, tag="mis1", name=f"pp{lbl}{sl}", bufs=1)
                        nc.tensor.matmul(pp[:, :], sm[:, :], pv[:, :], start=True, stop=True)
                        ps_ = seqp.tile([128, 18], f32, tag=f"pv{lbl}{sl}", name=f"pv{lbl}{sl}")
                        nc.vector.tensor_copy(ps_[:, :], pp[:, :])
                        planes[lbl + sl] = ps_
                DXV = {0: "m", 1: "c", 2: "p"}

                def build_idxr(rowlbl, collbl, rl):
                    tnat = seqp.tile([128, 144], f32, tag=f"tnat{rl}", name=f"tnat{rl}")
                    for A in range(9):
                        dy, dx = A // 3, A % 3
                        rp = planes[rowlbl + DXV[dx]]
                        cp = planes[collbl + DXV[dx]]
                        rsl = slice(2 - dy, 2 - dy + 16)
                        nc.vector.scalar_tensor_tensor(tnat[:, 16 * A:16 * (A + 1)],
                                                       rp[:, rsl], 130.0, cp[:, rsl], OP.mult, OP.add)
                    nc.vector.tensor_tensor(tnat[:, :], tnat[:, :], ms_t[:, :], op=OP.mult)
                    nc.vector.tensor_tensor(tnat[:, :], tnat[:, :], fs_t[:, :], op=OP.add)
                    t1p = psp.tile([128, 128], f32, tag="rectp", name=f"t1p{rl}")
                    nc.tensor.transpose(t1p[:, :], tnat[:, 0:128], ident_sb[:, :])
                    t1s = seqp.tile([128, 128], f32, tag=f"t1s{rl}", name=f"t1s{rl}")
                    nc.vector.tensor_copy(t1s[:, :], t1p[:, :])
                    t2p = psp.tile([16, 128], f32, tag="mis1", name=f"t2p{rl}", bufs=1)
                    nc.tensor.transpose(t2p[:, :], tnat[:, 128:144], ident_sb[:, :])
                    t2s = seqp.tile([16, 128], f32, tag=f"t2s{rl}", name=f"t2s{rl}")
                    nc.vector.tensor_copy(t2s[:, :], t2p[:, :])
                    idxs16 = seqp.tile([16, 1152], i16, tag=f"ix{rl}", name=f"ix{rl}")
                    iv = idxs16[:, :].rearrange("q (f ph) -> q f ph", ph=8)
                    for ph in range(8):
                        qp = psp.tile([16, 128], f32, tag="mis1", name=f"qp{rl}{ph}", bufs=1)
                        nc.tensor.transpose(qp[:, :], t1s[:, 16 * ph:16 * (ph + 1)], ident_sb[:, :])
                        nc.vector.tensor_copy(iv[:, 0:128, ph], qp[:, :])
                        qp2 = psp.tile([16, 16], f32, tag="mis1", name=f"qp2{rl}{ph}", bufs=1)
                        nc.tensor.transpose(qp2[:, :], t2s[:, 16 * ph:16 * (ph + 1)], ident_sb[0:16, 0:16])
                        nc.vector.tensor_copy(iv[:, 128:144, ph], qp2[:, :])
                    idxr = workp.tile([128, 1152], i16, tag="ixr", name=f"ixr{rl}")
                    nc.sync.dma_start(idxr[0:16, :], idxs16[:, :])
                    nc.sync.dma_start(idxr[16:32, :], idxr[0:16, :])
                    nc.sync.dma_start(idxr[32:64, :], idxr[0:32, :])
                    nc.sync.dma_start(idxr[64:128, :], idxr[0:64, :])
                    return idxr

                def reconstruct(pad_tab, rowfirst, blend_slab, out_tensor, rl):
                    idxr = build_idxr("y" if rowfirst else "x", "x" if rowfirst else "y", rl)
                    acc = bigp.tile([128, ROWS * 256], f32, tag="acc", name=f"acc{rl}")
                    nc.vector.memset(acc[:, :], 0.0)
                    in_ap = bass.AP(pad_tab.ap().tensor, 0, [[256, NPAD], [1, 256]])
                    for ch in range(18):
                        uh = ch % 2
                        g = workp.tile([128, 8, 256], f32, tag="recg", name=f"recg{rl}{ch}")
                        nc.gpsimd.dma_gather(out_ap=g[:, :, :], in_ap=in_ap,
                                             idxs_ap=idxr[:, 64 * ch:64 * (ch + 1)],
                                             num_idxs=1024, num_idxs_reg=1024,
                                             elem_size=256, elem_step=256)
                        asl = acc[:, 2048 * uh: 2048 * (uh + 1)]
                        nc.vector.tensor_tensor(asl, asl, g[:, :, :].rearrange("p a c -> p (a c)"),
                                                op=OP.add)
                    for u in range(ROWS):
                        nc.vector.tensor_scalar(acc[:, 256 * u:256 * (u + 1)],
                                                acc[:, 256 * u:256 * (u + 1)],
                                                invcs[:, u:u + 1], None, op0=OP.mult)
                    for half in range(2):
                        ssc = workp.tile([128, 2048], f32, tag="ssc", name=f"ssc{rl}{half}", bufs=1)
                        nc.sync.dma_start(ssc[:, :], blend_slab[128 * half:128 * (half + 1), :, :]
                                          .rearrange("c r w -> c (r w)"))
                        nc.vector.tensor_scalar(ssc[:, :], ssc[:, :], C04, None, op0=OP.mult)
                        for u in range(ROWS):
                            tp_ = psp.tile([128, 128], f32, tag="rectp", name=f"rtp{rl}{half}{u}")
                            nc.tensor.transpose(tp_[:, :],
                                                acc[:, u * 256 + half * 128: u * 256 + half * 128 + 128],
                                                ident_sb[:, :])
                            nc.vector.tensor_tensor(ssc[:, 128 * u:128 * (u + 1)], tp_[:, :],
                                                    ssc[:, 128 * u:128 * (u + 1)], op=OP.add)
                        nc.sync.dma_start(out_tensor[128 * half:128 * (half + 1), :, :]
                                          .rearrange("c r w -> c (r w)"), ssc[:, :])

                reconstruct(tpad, True, s_slab, rec_s_out, "s")
                reconstruct(spad, False, t_slab, rec_t_out, "t")

    nc.compile()
    return nc


_CACHE = {}


def _get_nc():
    if "nc" not in _CACHE:
        _CACHE["nc"] = _build()
    return _CACHE["nc"]


def _make_in_maps(source_feat, target_feat, init_nnf_fwd, init_nnf_bwd, rand_fwd, rand_bwd):
    sf = np.asarray(source_feat, np.float32)
    tf = np.asarray(target_feat, np.float32)
    nf = np.asarray(init_nnf_fwd, np.float32)
    nb = np.asarray(init_nnf_bwd, np.float32)
    rf = np.asarray(rand_fwd, np.float32)
    rb = np.asarray(rand_bwd, np.float32)
    tpad = _pad_hwc(tf[0])
    spad = _pad_hwc(sf[0])
    band = _make_band()
    m16 = _make_m16()
    shp = _make_shift(1)
    shm = _make_shift(-1)
    ident = np.eye(128, dtype=np.float32)
    maps = []
    for k in range(N_CORES):
        rs = slice(k * ROWS, (k + 1) * ROWS)
        M_, F_ = _make_mf(k)
        maps.append({
            "s_slab": np.ascontiguousarray(sf[0][:, rs, :]),
            "t_slab": np.ascontiguousarray(tf[0][:, rs, :]),
            "tpad": tpad, "spad": spad,
            "nnf_f": _nnf_slab(nf, k), "nnf_b": _nnf_slab(nb, k),
            "rand_f": _rand_slab(rf, k), "rand_b": _rand_slab(rb, k),
            "hsel": _make_hsel(k), "minp": M_, "finp": F_,
            "invcnt": _make_invcnt(k),
            "band": band, "m16": m16, "shp": shp, "shm": shm, "ident": ident,
        })
    return maps


def kernel(source_feat, target_feat, init_nnf_fwd, init_nnf_bwd, rand_fwd, rand_bwd):
    nc = _get_nc()
    maps = _make_in_maps(source_feat, target_feat, init_nnf_fwd, init_nnf_bwd,
                         rand_fwd, rand_bwd)
    res = run_bass_kernel_spmd(nc, maps, list(range(N_CORES)))
    rec_s = np.concatenate([res.results[k]["rec_s"] for k in range(N_CORES)], axis=1)[None]
    rec_t = np.concatenate([res.results[k]["rec_t"] for k in range(N_CORES)], axis=1)[None]
    sim = np.concatenate([res.results[k]["sim"] for k in range(N_CORES)], axis=0)[None]
    return rec_s.astype(np.float32), rec_t.astype(np.float32), sim.astype(np.float32)
